# revision 15
# baseline (speedup 1.0000x reference)
"""DeepConvNet Trainium2 kernel.

3x [Conv3x3(pad=1) -> ReLU -> MaxPool2x2] -> Linear, N=64, input 3x128x128.

Sharding: pure data parallel, 8 images per NeuronCore across 8 cores.

Per-core dataflow (all activations bf16 in SBUF, fp32 PSUM accumulation):
  conv1: im2col in partitions. 4-image groups, block-diagonal weights:
         K = 9 taps x 3 ch x 4 imgs = 108 partitions, M = 4 imgs x 32 ch.
         rhs built by 9 strided DMAs from host-padded x (HBM).
  pool:  DVE tensor_max psum col-pairs -> row-pairs, written into a
         zero-bordered padded tile; ScalarE applies bias+ReLU in place.
  conv2: kx-replicated rhs (K = 32 ch x 3 kx = 96) built by one SBUF->SBUF
         DMA per image; 3 accumulated matmuls over ky; 2 images run
         concurrently via column tiling (img A -> array cols 0-63,
         img B -> cols 64-127).
  conv3: no replication: 9 accumulated matmuls (K=64) per image; 2 images
         run concurrently via row tiling (img A rows 0-63, B rows 64-127).
  fc:    256 accumulated matmuls (K=128 channels, one per spatial p),
         N = 8 images, M = 10 classes.
"""

import os
import sys

import numpy as np

for _p in ("/opt/trn_rl_repo", "/root/.axon_site/_ro/trn_rl_repo"):
    if os.path.isdir(_p) and _p not in sys.path:
        sys.path.insert(0, _p)

import ml_dtypes

import concourse.bass as bass
import concourse.mybir as mybir
import concourse.tile as tile
from concourse import bacc
from concourse.bass_utils import run_bass_kernel_spmd

BF16 = mybir.dt.bfloat16
F32 = mybir.dt.float32
NPBF16 = ml_dtypes.bfloat16

N_CORES = 8
IMGS = 8          # images per core
GROUPS = 2        # conv1 image groups per core (4 imgs each)
G1 = 130          # conv1 padded width/height
W1WIN = 127 * G1 + 128  # flat window length per conv1 im2col row
W1ALLOC = 128 * G1
P1 = 66           # conv1 pooled padded grid (64 + 2)
P1F = 67 * 66     # pp1 alloc free size (one guard row)
P2 = 34           # conv2 pooled padded grid (32 + 2)
P2F = 34 * 34


def _build_nc(dma_engine="sync", dbg=False):
    nc = bacc.Bacc("TRN2", target_bir_lowering=False, debug=False)

    xp = nc.dram_tensor("xp", [IMGS * 3 * G1 * G1], BF16, kind="ExternalInput")
    lhsT1 = nc.dram_tensor("lhsT1", [108, 128], BF16, kind="ExternalInput")
    lhsT2 = nc.dram_tensor("lhsT2", [96, 3 * 128], BF16, kind="ExternalInput")
    lhsT3 = nc.dram_tensor("lhsT3", [128, 9 * 128], BF16, kind="ExternalInput")
    wfc = nc.dram_tensor("wfc", [128, 2560], BF16, kind="ExternalInput")
    b1d = nc.dram_tensor("b1d", [128, 1], F32, kind="ExternalInput")
    b2d = nc.dram_tensor("b2d", [128, 1], F32, kind="ExternalInput")
    b3d = nc.dram_tensor("b3d", [128, 1], F32, kind="ExternalInput")
    bfcd = nc.dram_tensor("bfcd", [10, 1], F32, kind="ExternalInput")
    scores = nc.dram_tensor("scores", [10, 8], F32, kind="ExternalOutput")

    Relu = mybir.ActivationFunctionType.Relu
    Ident = mybir.ActivationFunctionType.Identity
    Copy = mybir.ActivationFunctionType.Copy

    with tile.TileContext(nc) as tc:
        dma = getattr(nc, dma_engine)
        with (
            tc.tile_pool(name="wts", bufs=1) as wp,
            tc.tile_pool(name="rhs1", bufs=2) as rhs1p,
            tc.tile_pool(name="pp1", bufs=2) as pp1p,
            tc.tile_pool(name="rhs2", bufs=4) as rhs2p,
            tc.tile_pool(name="pp2", bufs=4) as pp2p,
            tc.tile_pool(name="xall", bufs=1) as xallp,
            tc.tile_pool(name="tmp", bufs=6) as tmpp,
            tc.tile_pool(name="ps", bufs=4, space="PSUM") as psp,
        ):
            # ---- weights into SBUF ----
            t_l1 = wp.tile([108, 128], BF16)
            dma.dma_start(out=t_l1[:], in_=lhsT1.ap())
            t_l2 = wp.tile([96, 3 * 128], BF16)
            dma.dma_start(out=t_l2[:], in_=lhsT2.ap())
            t_l3 = wp.tile([128, 9 * 128], BF16)
            dma.dma_start(out=t_l3[:], in_=lhsT3.ap())
            t_wfc = wp.tile([128, 2560], BF16)
            dma.dma_start(out=t_wfc[:], in_=wfc.ap())
            t_b1 = wp.tile([128, 1], F32)
            dma.dma_start(out=t_b1[:], in_=b1d.ap())
            t_b2 = wp.tile([128, 1], F32)
            dma.dma_start(out=t_b2[:], in_=b2d.ap())
            t_b3 = wp.tile([128, 1], F32)
            dma.dma_start(out=t_b3[:], in_=b3d.ap())
            t_bfc = wp.tile([10, 1], F32)
            dma.dma_start(out=t_bfc[:], in_=bfcd.ap())

            x_all = xallp.tile([128, 2048], BF16)

            def pool_psum(ps, out_ap, w, name):
                """2x2 maxpool a [128, 1024] psum tile holding rows of width w.

                PSUM can only feed one operand of a DVE tensor op, so ScalarE
                first copies the even columns to SBUF; DVE then maxes the odd
                PSUM columns against that copy, and finally maxes row pairs
                into out_ap (free dims (1024/w/2, w/2))."""
                psv = ps.rearrange("p (a two) -> p a two", two=2)
                cp = tmpp.tile([128, 512], F32, tag="tmpc", name=f"cp_{name}")
                nc.scalar.activation(cp[:], psv[:, :, 0], Copy)
                m1 = tmpp.tile([128, 512], BF16, tag="tmpm", name=f"m1_{name}")
                nc.vector.tensor_max(m1[:], psv[:, :, 1], cp[:])
                tv = m1.rearrange("p (y two x) -> p y two x", two=2, x=w // 2)
                nc.vector.tensor_max(out_ap, tv[:, :, 0, :], tv[:, :, 1, :])

            # =======================  conv1  =======================
            pp1_tiles = []
            for g in range(GROUPS):
                rhs1 = rhs1p.tile([108, W1ALLOC], BF16, tag="rhs1")
                for t in range(9):
                    a, b = divmod(t, 3)
                    src = bass.AP(
                        xp,
                        g * 4 * 3 * G1 * G1 + a * G1 + b,
                        [[3 * G1 * G1, 4], [G1 * G1, 3], [1, W1WIN]],
                    )
                    dma.dma_start(out=rhs1[12 * t : 12 * t + 12, 0:W1WIN], in_=src)
                if dbg and g == 0:
                    d_rhs1 = nc.dram_tensor(
                        "d_rhs1", [108, W1ALLOC], BF16, kind="ExternalOutput"
                    )
                    dma.dma_start(out=d_rhs1.ap(), in_=rhs1[:])
                rhs1v = rhs1.rearrange("p (y x) -> p y x", x=G1)

                pp1 = pp1p.tile([128, P1F], BF16, tag="pp1")
                nc.gpsimd.memset(pp1[:], 0)

                for k in range(16):
                    ps = psp.tile([128, 1024], F32, tag="ps")
                    for h in range(2):
                        y0 = k * 8 + h * 4
                        nc.tensor.matmul(
                            ps[:, h * 512 : (h + 1) * 512],
                            t_l1[:],
                            rhs1v[:, y0 : y0 + 4, 0:128],
                            start=True,
                            stop=True,
                        )
                    # maxpool 2x2 on psum [128, 1024] = 8 rows x 128
                    pv = pp1.rearrange("p (r q) -> p r q", q=P1)
                    Y0 = k * 4
                    pool_psum(ps, pv[:, Y0 + 1 : Y0 + 5, 1:65], 128, f"c1_{g}_{k}")
                pv = pp1.rearrange("p (r q) -> p r q", q=P1)
                interior = pv[:, 1:65, 1:65]
                nc.scalar.activation(interior, interior, Relu, bias=t_b1[:, 0:1])
                pp1_tiles.append(pp1)

            # =======================  conv2  =======================
            pp2_tiles = []
            for q in range(4):  # image pairs
                g, pr = divmod(q, 2)
                pp1 = pp1_tiles[g]
                p1pitch = pp1.ap[0][0]
                rhs2 = []
                for j in range(2):  # imgs 2q+j; within-group index pr*2+j
                    i1 = pr * 2 + j
                    r2 = rhs2p.tile([96, 66 * 66], BF16, tag="rhs2")
                    r2pitch = r2.ap[0][0]
                    for kx in range(3):
                        src = bass.AP(
                            pp1.tensor,
                            (32 * i1) * p1pitch + kx,
                            [[p1pitch, 32], [1, 66 * 66]],
                        )
                        dst = bass.AP(
                            r2.tensor,
                            r2.offset + kx * r2pitch,
                            [[3 * r2pitch, 32], [1, 66 * 66]],
                        )
                        dma.dma_start(out=dst, in_=src)
                    if dbg and q == 0 and j == 0:
                        d_rhs2 = nc.dram_tensor(
                            "d_rhs2", [96, 66 * 66], BF16, kind="ExternalOutput"
                        )
                        dma.dma_start(out=d_rhs2.ap(), in_=r2[:])
                    rhs2.append(r2.rearrange("p (r q) -> p r q", q=66))

                pp2 = pp2p.tile([128, P2F], BF16, tag="pp2")
                nc.gpsimd.memset(pp2[:], 0)

                for k in range(4):
                    ps = psp.tile([128, 1024], F32, tag="ps")
                    for h in range(2):
                        Y0 = k * 16 + h * 8
                        for ky in range(3):
                            for j in range(2):
                                nc.tensor.matmul(
                                    ps[64 * j : 64 * j + 64, h * 512 : (h + 1) * 512],
                                    t_l2[:, ky * 128 + 64 * j : ky * 128 + 64 * j + 64],
                                    rhs2[j][:, Y0 + ky : Y0 + ky + 8, 0:64],
                                    start=(ky == 0),
                                    stop=(ky == 2),
                                )
                    pv = pp2.rearrange("p (r q) -> p r q", q=P2)
                    Y0 = k * 8
                    pool_psum(ps, pv[:, Y0 + 1 : Y0 + 9, 1:33], 64, f"c2_{q}_{k}")
                pv = pp2.rearrange("p (r q) -> p r q", q=P2)
                interior = pv[:, 1:33, 1:33]
                nc.scalar.activation(interior, interior, Relu, bias=t_b2[:, 0:1])
                pp2_tiles.append(pp2)

            # =======================  conv3  =======================
            for q in range(4):
                pp2 = pp2_tiles[q]
                pv2 = pp2.rearrange("p (r q) -> p r q", q=P2)
                ps_ab = [
                    psp.tile([128, 1024], F32, tag="ps", name=f"ps3_{q}_{jj}")
                    for jj in range(2)
                ]
                for h in range(2):
                    Y0 = h * 16
                    for t in range(9):
                        a, b = divmod(t, 3)
                        for j in range(2):  # img A (rows 0-63), img B (rows 64-127)
                            nc.tensor.matmul(
                                ps_ab[j][:, h * 512 : (h + 1) * 512],
                                t_l3[64 * j : 64 * j + 64, t * 128 : (t + 1) * 128],
                                pv2[64 * j : 64 * j + 64, Y0 + a : Y0 + a + 16, b : b + 32],
                                start=(t == 0),
                                stop=(t == 8),
                            )
                for j in range(2):
                    img = 2 * q + j
                    xv = x_all.rearrange("p (i q) -> p i q", q=256)
                    ov = xv[:, img, :].rearrange("p (y x) -> p y x", x=16)
                    pool_psum(ps_ab[j], ov, 32, f"c3_{q}_{j}")

            nc.scalar.activation(x_all[:], x_all[:], Relu, bias=t_b3[:, 0:1])

            # =======================  fc  =======================
            ps_fc = psp.tile([10, 8], F32, tag="ps")
            xv = x_all.rearrange("p (i q) -> p i q", q=256)
            for p in range(256):
                nc.tensor.matmul(
                    ps_fc[:],
                    t_wfc[:, 10 * p : 10 * p + 10],
                    xv[:, :, p],
                    start=(p == 0),
                    stop=(p == 255),
                )
            sc = wp.tile([10, 8], F32)
            nc.scalar.activation(sc[:], ps_fc[:], Ident, bias=t_bfc[:, 0:1])
            dma.dma_start(out=scores.ap(), in_=sc[:])

            if dbg:
                d_pp1 = nc.dram_tensor("d_pp1", [128, P1F], BF16, kind="ExternalOutput")
                dma.dma_start(out=d_pp1.ap(), in_=pp1_tiles[0][:])
                d_pp2 = nc.dram_tensor("d_pp2", [128, P2F], BF16, kind="ExternalOutput")
                dma.dma_start(out=d_pp2.ap(), in_=pp2_tiles[0][:])
                d_xall = nc.dram_tensor("d_xall", [128, 2048], BF16, kind="ExternalOutput")
                dma.dma_start(out=d_xall.ap(), in_=x_all[:])

    nc.compile()
    return nc


def _prep_weights(w1, b1, w2, b2, w3, b3, w_fc, b_fc):
    """Host-side weight rearrangement (shared across cores)."""
    # conv1 block-diagonal lhsT: rows p = t*12 + img*3 + c, cols m = img*32 + f
    l1 = np.zeros((108, 128), np.float32)
    for t in range(9):
        a, b = divmod(t, 3)
        for img in range(4):
            # w1[f, c, a, b] at [t*12 + img*3 + c, img*32 + f]
            l1[t * 12 + img * 3 : t * 12 + img * 3 + 3, img * 32 : img * 32 + 32] = (
                w1[:, :, a, b].T
            )
    # conv2: rows p = c*3 + kx, col block ky: [W2_ky | W2_ky]
    l2 = np.zeros((96, 3 * 128), np.float32)
    for ky in range(3):
        for kx in range(3):
            # w2[f, c, ky, kx] -> rows c*3+kx
            blk = w2[:, :, ky, kx].T  # [c, f]
            l2[kx::3, ky * 128 : ky * 128 + 64] = blk
            l2[kx::3, ky * 128 + 64 : ky * 128 + 128] = blk
    # conv3: rows c (dup at 64+c), col block t
    l3 = np.zeros((128, 9 * 128), np.float32)
    for t in range(9):
        a, b = divmod(t, 3)
        blk = w3[:, :, a, b].T  # [c=64, f=128]
        l3[0:64, t * 128 : (t + 1) * 128] = blk
        l3[64:128, t * 128 : (t + 1) * 128] = blk
    # fc: w_fc[c*256 + p, cls] -> wfc[c, p*10 + cls]
    wf = np.ascontiguousarray(
        w_fc.reshape(128, 256, 10).transpose(0, 1, 2).reshape(128, 2560)
    )
    return {
        "lhsT1": l1.astype(NPBF16),
        "lhsT2": l2.astype(NPBF16),
        "lhsT3": l3.astype(NPBF16),
        "wfc": wf.astype(NPBF16),
        "b1d": np.tile(np.asarray(b1, np.float32), 4).reshape(128, 1),
        "b2d": np.tile(np.asarray(b2, np.float32), 2).reshape(128, 1),
        "b3d": np.asarray(b3, np.float32).reshape(128, 1),
        "bfcd": np.asarray(b_fc, np.float32).reshape(10, 1),
    }


_NC_CACHE = {}


def get_nc():
    if "nc" not in _NC_CACHE:
        _NC_CACHE["nc"] = _build_nc()
    return _NC_CACHE["nc"]


def kernel(x, w1, b1, w2, b2, w3, b3, w_fc, b_fc, **run_kwargs):
    x = np.asarray(x, np.float32)
    wts = _prep_weights(
        np.asarray(w1, np.float32), np.asarray(b1, np.float32),
        np.asarray(w2, np.float32), np.asarray(b2, np.float32),
        np.asarray(w3, np.float32), np.asarray(b3, np.float32),
        np.asarray(w_fc, np.float32), np.asarray(b_fc, np.float32),
    )
    xpad = np.pad(x, ((0, 0), (0, 0), (1, 1), (1, 1))).astype(NPBF16)
    in_maps = []
    for core in range(N_CORES):
        m = dict(wts)
        m["xp"] = np.ascontiguousarray(xpad[core * IMGS : (core + 1) * IMGS]).reshape(-1)
        in_maps.append(m)

    nc = get_nc()
    res = run_bass_kernel_spmd(nc, in_maps, core_ids=list(range(N_CORES)), **run_kwargs)
    out = np.concatenate([r["scores"].T for r in res.results], axis=0)
    kernel.last_results = res
    return out.astype(np.float32)
